# revision 17
# baseline (speedup 1.0000x reference)
"""Bidirectional chamfer distance (nn_DisplacementLoss) on 8 trn2 NeuronCores.

Sharding: 8 cores = 4 batches x 2 directions. Core c handles batch c%4,
direction c//4 (0: pred->gt, 1: gt->pred). Each core computes the row-mins
of its 5000x5000 squared-distance matrix via a K=5 augmented fp32r matmul
(d2 = |x|^2 + |y|^2 - 2<x,y> folded into one contraction) tiled 128x512
into PSUM.

Min-reduction pipeline per 128-row tile: half the m-chunks are consumed by
DVE straight from PSUM, the other half are copied PSUM->SBUF by the Scalar
engine; DVE pairs one PSUM group with one SBUF group per tensor_tensor_reduce
(out = min(in0,in1), accum = running min) so it consumes 2 elements/cycle.
"""

import numpy as np

B, N, D = 4, 5000, 3
NP = 5120  # padded pred points: 40 tiles x 128 partitions
MP = 5120  # padded gt points: 10 chunks x 512
NT = NP // 128
K = 5  # augmented contraction: [-2x0,-2x1,-2x2, x2, 1] . [y0,y1,y2, 1, y2sum]
BIG = 1.0e30

_compiled = None
_ttmin_op = None


def _register_tt_min_reduce():
    """Custom DVE op: out = min(in0,in1); accum_out = min(s0, min_k out[k]).
    2-input 1x DVE op (rd0+rd1) - consumes two streams per cycle while
    producing the running row-min in accum_out."""
    global _ttmin_op
    if _ttmin_op is not None:
        return _ttmin_op
    import concourse.dve_ops as dops
    from concourse.dve_spec import Spec, Src0, Src1, C0, minn, _has_src1, lower
    from concourse.dve_uop import DveOpSpec

    for op in dops.OPS:
        if op.name == "TT_MIN_REDUCE_ANT":
            _ttmin_op = op
            return op

    def _ref(in0, in1, c0, c1, c2):
        b = np.minimum(in0.astype(np.float32), in1.astype(np.float32))
        acc = np.minimum(
            np.asarray(c0, dtype=np.float32),
            b.reshape(b.shape[0], -1).min(axis=-1, keepdims=True),
        ).astype(np.float32)
        return b, acc

    spec = Spec(body=minn(Src0, Src1), accum=minn, accum_init=C0, reference=_ref)
    op = dops.DveOp("TT_MIN_REDUCE_ANT", spec, subdim=False, uops_sha={})
    dops.OPS.append(op)
    dops.CUSTOM_DVE_SPECS[op.name] = spec
    row = dops._CUSTOM_DVE_ROW_BASE + len(dops.OPS) - 1
    assert row < 0x20
    dops._SUB_OPCODE_FOR_NAME[op.name] = row
    for ver in ("v3", "v4"):
        tmp = DveOpSpec(
            name=op.name, opcode=row, uops=lower(spec, ver=ver),
            rd1_en=_has_src1(spec),
        )
        op.uops_sha[ver] = tmp.sha(ver)
    _ttmin_op = op
    return op


def _build_program(repeat=None, mode="tail_dve", big_bufs=False):
    import contextlib

    import concourse.bacc as bacc
    import concourse.tile as tile
    import concourse.mybir as mybir

    f32 = mybir.dt.float32
    f32r = mybir.dt.float32r
    ttmin = _register_tt_min_reduce()
    nc = bacc.Bacc(debug=False, num_devices=8)
    a_dram = nc.dram_tensor("a_aug", [K, NP], f32r, kind="ExternalInput").ap()
    b_dram = nc.dram_tensor("b_aug", [K, MP], f32r, kind="ExternalInput").ap()
    out_dram = nc.dram_tensor("minvals", [128, NT], f32, kind="ExternalOutput").ap()

    # Per n-tile the 5120-wide m-row is processed as 3 (dve, act) group
    # pairs: the dve group stays in PSUM (TTR in0), the act group is copied
    # to SBUF by ScalarE (TTR in1). Group widths 1024,1024,512.
    pairs = [(0, 1024), (2048, 1024), (4096, 512)]  # (dve group offset, width)

    merged = mode in ("merged", "paired25")
    pa_bufs = 1 if merged else 2
    stage_bufs = 4 if big_bufs else 3
    scratch_bufs = 3 if big_bufs else 2
    acc_bufs = 3 if big_bufs else 2
    with tile.TileContext(nc) as tc:
        with (
            tc.tile_pool(name="const", bufs=1) as const_pool,
            tc.tile_pool(name="acc", bufs=acc_bufs) as acc_pool,
            tc.tile_pool(name="stage", bufs=stage_bufs) as stage_pool,
            tc.tile_pool(name="scratch", bufs=scratch_bufs) as scratch_pool,
            tc.tile_pool(name="psum_d", bufs=2, space="PSUM") as psum_d_pool,
            tc.tile_pool(name="psum_a", bufs=pa_bufs, space="PSUM") as psum_a_pool,
        ):
            a_sb = const_pool.tile([K, NP], f32r)
            nc.sync.dma_start(a_sb[:], a_dram[:])
            b_sb = const_pool.tile([K, MP], f32r)
            nc.sync.dma_start(b_sb[:], b_dram[:])
            out_sb = const_pool.tile([128, NT], f32)
            bigs = const_pool.tile([128, 1024], f32)
            nc.vector.memset(bigs[:], BIG)

            # Optional benchmark mode: repeat the (idempotent) compute body
            # R times inside a dynamic loop so per-iteration HW time can be
            # measured from the wall-clock slope between two R values.
            rep_ctx = (
                tc.For_i(0, repeat, 1) if repeat is not None else contextlib.nullcontext()
            )
            with rep_ctx:
                _emit_body(
                    nc, tile, mybir, ttmin, pairs,
                    a_sb, b_sb, out_sb, bigs,
                    acc_pool, stage_pool, scratch_pool, psum_d_pool, psum_a_pool,
                    mode,
                )
            nc.sync.dma_start(out_dram[:], out_sb[:])

    nc.compile()
    return nc


def _emit_body(nc, tile, mybir, ttmin, pairs, a_sb, b_sb, out_sb, bigs,
               acc_pool, stage_pool, scratch_pool, psum_d_pool, psum_a_pool,
               mode):
    f32 = mybir.dt.float32

    def mm(ptile, lhsT, m0, w):
        for i in range(w // 512):
            nc.tensor.matmul(
                ptile[:, 512 * i : 512 * (i + 1)],
                lhsT,
                b_sb[:, m0 + 512 * i : m0 + 512 * (i + 1)],
                start=True,
                stop=True,
            )

    def ttr(scr_w, in0, in1, s0, acc_out):
        scr = scratch_pool.tile([128, 1024], f32, name="scr")
        nc.vector._custom_dve(
            ttmin, out=scr[:, :scr_w], in0=in0, in1=in1, s0=s0, accum_out=acc_out
        )

    if mode in ("merged", "paired25"):
        for nt in range(NT):
            lhsT = a_sb[:, nt * 128 : (nt + 1) * 128]
            out_col = out_sb[:, nt : nt + 1]
            # act-side: one contiguous 2048 group, single ScalarE copy
            pa = psum_a_pool.tile([128, 2048], f32, name="pabig")
            mm(pa, lhsT, 2048, 2048)
            stagedA = stage_pool.tile([128, 2048], f32, name="stA")
            nc.scalar.copy(stagedA[:], pa[:])
            # dve-side groups
            pd0 = psum_d_pool.tile([128, 1024], f32, name="pg0")
            mm(pd0, lhsT, 0, 1024)
            pd1 = psum_d_pool.tile([128, 1024], f32, name="pg0")
            mm(pd1, lhsT, 1024, 1024)
            acc1 = acc_pool.tile([128, 1], f32, name="acc")
            ttr(1024, pd0[:], stagedA[:, :1024], BIG, acc1)
            acc2 = acc_pool.tile([128, 1], f32, name="acc")
            ttr(1024, pd1[:], stagedA[:, 1024:2048], acc1, acc2)
            if mode == "merged":
                pdt = psum_d_pool.tile([128, 1024], f32, name="pg0")
                mm(pdt, lhsT, 4096, 1024)
                ttr(1024, pdt[:], bigs[:, :1024], acc2, out_col)
            else:
                pdt = psum_d_pool.tile([128, 1024], f32, name="pg0")
                mm(pdt, lhsT, 4096, 512)
                pat = psum_d_pool.tile([128, 1024], f32, name="pg0")
                mm(pat, lhsT, 4608, 512)
                staged2 = stage_pool.tile([128, 512], f32, name="st2")
                nc.scalar.copy(staged2[:], pat[:, :512])
                ttr(512, pdt[:, :512], staged2[:], acc2, out_col)
        return

    for nt in range(NT):
        lhsT = a_sb[:, nt * 128 : (nt + 1) * 128]
        prev_acc = None

        if mode == "noact":
            # every 1024-group consumed directly by DVE (in1 = BIG consts)
            groups = [(0, 1024), (1024, 1024), (2048, 1024), (3072, 1024), (4096, 1024)]
        elif mode == "tail_dve":
            groups = pairs[:2] + [("tail", 1024)]
        elif mode == "tail_first":
            # ACT-independent direct group first: DVE never waits for the
            # staged copy at tile start
            groups = [("tail", 1024)] + pairs[:2]
        else:
            groups = pairs

        n_groups = len(groups)
        for pi, (d0, w) in enumerate(groups):
            last = pi == n_groups - 1
            acc_out = out_sb[:, nt : nt + 1] if last else acc_pool.tile([128, 1], f32)
            scratch = scratch_pool.tile([128, 1024], f32)

            if mode == "noact":
                pool = psum_d_pool if pi % 2 == 0 else psum_a_pool
                pd = pool.tile([128, 1024], f32, name=f"pg{pi % 2}")
                mm(pd, lhsT, d0, w)
                in1 = bigs[:, :w]
            elif d0 == "tail":
                pd = psum_d_pool.tile([128, 1024], f32, name="pg0")
                mm(pd, lhsT, 4096, w)
                in1 = bigs[:, :w]
            else:
                a0 = d0 + w  # act group sits right after the dve group
                pd = psum_d_pool.tile([128, 1024], f32, name="pg0")
                pa = psum_a_pool.tile([128, 1024], f32, name="pg1")
                for i in range(w // 512):
                    nc.tensor.matmul(
                        pd[:, 512 * i : 512 * (i + 1)], lhsT,
                        b_sb[:, d0 + 512 * i : d0 + 512 * (i + 1)],
                        start=True, stop=True,
                    )
                    nc.tensor.matmul(
                        pa[:, 512 * i : 512 * (i + 1)], lhsT,
                        b_sb[:, a0 + 512 * i : a0 + 512 * (i + 1)],
                        start=True, stop=True,
                    )
                staged = stage_pool.tile([128, 1024], f32)
                nc.scalar.copy(staged[:, :w], pa[:, :w])
                in1 = staged[:, :w]

            nc.vector._custom_dve(
                ttmin,
                out=scratch[:, :w],
                in0=pd[:, :w],
                in1=in1,
                s0=BIG if prev_acc is None else prev_acc,
                accum_out=acc_out,
            )
            prev_acc = acc_out


def _get_program():
    global _compiled
    if _compiled is None:
        _compiled = _build_program()
    return _compiled


def _make_core_inputs(x, y):
    """x: query points [N,3] f32, y: database points [N,3] f32.
    Returns (a_aug [K,NP], b_aug [K,MP]) so that (a_aug.T @ b_aug)[n,m] = d2."""
    a = np.zeros((K, NP), dtype=np.float32)
    a[0:3, :N] = -2.0 * x.T
    a[3, :N] = (x * x).sum(axis=1)
    a[4, :N] = 1.0
    b = np.zeros((K, MP), dtype=np.float32)
    b[0:3, :N] = y.T
    b[3, :N] = 1.0
    b[4, :N] = (y * y).sum(axis=1)
    b[4, N:] = BIG  # padded columns never win the min
    return a, b


def _run(pred_samples, gt_samples, trace=False):
    from concourse.bass_utils import run_bass_kernel_spmd

    nc = _get_program()
    pred = np.asarray(pred_samples, dtype=np.float32)
    gt = np.asarray(gt_samples, dtype=np.float32)
    in_maps = []
    for c in range(8):
        bidx = c % 4
        if c < 4:
            a, bb = _make_core_inputs(pred[bidx], gt[bidx])
        else:
            a, bb = _make_core_inputs(gt[bidx], pred[bidx])
        in_maps.append({"a_aug": a, "b_aug": bb})
    res = run_bass_kernel_spmd(nc, in_maps, list(range(8)), trace=trace)
    return res


def _gather(res):
    total = 0.0
    for c in range(8):
        mv = res.results[c]["minvals"]  # [128, NT]
        mins = mv.transpose(1, 0).reshape(-1)[:N].astype(np.float64)
        mins = np.maximum(mins, 0.0)
        total += mins.mean()
    return np.float32(total / 4.0)


def kernel(pred_samples, gt_samples):
    res = _run(pred_samples, gt_samples)
    return _gather(res)
